# revision 39
# baseline (speedup 1.0000x reference)
"""GQA attention forward on 8 TRN2 NeuronCores, tensor-parallel across heads.

Problem (hardcoded): B=2, T=2048, D=2048, 16 q-heads, 4 kv-heads, head_dim=128,
RoPE (rotate-half pairing i <-> i+64), causal softmax, output projection.

Sharding (per core c of 8):
  q-heads 2c, 2c+1 (rows 256c:256c+256 of wq), kv-head c//2 (rows of wk/wv),
  wo input-dim slice [:, 256c:256c+256]. x replicated. Each core computes a
  full-shape partial of the output (y_local @ wo_slice.T); host sums partials.

On-core layout: activations kept feature-major (qT/kT = [head_dim, tokens]) so
every matmul contraction dim lands on SBUF partitions with zero transposes of
x (host pre-transposes x once). Scores are computed transposed (ST[j,i]) so
P@V needs no transpose either. All matmul operands are fp16 (full PE speed,
half the LDWEIGHTS/DMA/SBUF cost of fp32r, ~5e-4 matmul rel err); PSUM
accumulation stays fp32. Max-subtraction is skipped (scores are O(5),
exp(s) < 100 is fp16-safe). Softmax denominators: est tiles are summed on the
vector engine (dsum) and reduced over partitions once per block by a
ones-MATRIX matmul whose output rows are the broadcast row-sum, so no
partition broadcast is needed; 1/d uses reciprocal_approx_fast. The two local
heads interleave tile-by-tile; each block's normalize is sandwiched after a
4-chunk prefix of the previous block's out-proj so the PE never waits on the
denominator-add chain, and recip/ymul retire before the next block needs the
ps_y banks. Weight loads are chunked per kt and issued from the gpsimd queue
(consts) in parallel with x tiles on sync; PSUM->SBUF drains alternate
Act/DVE so neither in-order queue serializes the pipeline.
"""
import math
import numpy as np

P = 128
B = 2
T = 2048
D = 2048
BT = B * T            # 4096
HD = 128              # head dim
QH = 2                # local q heads per core
KT = D // P           # 16 contraction tiles over D
NB = 512              # free-dim block (tokens) for matmuls
NBLK = BT // NB       # 8 bt blocks
IB = T // NB          # 4 i-blocks per batch
NCORES = 8
SCALE = 1.0 / math.sqrt(HD)

_CACHE = {}


def _build():
    import concourse.bass as bass
    import concourse.mybir as mybir
    from concourse import bacc
    from concourse.tile import TileContext

    F32 = mybir.dt.float32
    F16 = mybir.dt.float16
    EXP = mybir.ActivationFunctionType.Exp

    nc = bacc.Bacc("TRN2", target_bir_lowering=False, debug=False)

    xT_d = nc.dram_tensor("xT", [D, BT], F16, kind="ExternalInput").ap()
    wqT_d = nc.dram_tensor("wqT", [D, QH * HD], F16, kind="ExternalInput").ap()
    wkT_d = nc.dram_tensor("wkT", [D, HD], F16, kind="ExternalInput").ap()
    wvT_d = nc.dram_tensor("wvT", [D, HD], F16, kind="ExternalInput").ap()
    woT_d = nc.dram_tensor("woT", [QH * HD, D], F16, kind="ExternalInput").ap()
    cosT_d = nc.dram_tensor("cosT", [P, T], F16, kind="ExternalInput").ap()
    ssinT_d = nc.dram_tensor("ssinT", [P, T], F16, kind="ExternalInput").ap()
    permT_d = nc.dram_tensor("permT", [P, P], F16, kind="ExternalInput").ap()
    triu_d = nc.dram_tensor("triu", [P, P], F16, kind="ExternalInput").ap()
    ident_d = nc.dram_tensor("ident", [P, P], F16, kind="ExternalInput").ap()
    onesm_d = nc.dram_tensor("onesm", [P, P], F16, kind="ExternalInput").ap()
    out_d = nc.dram_tensor("out", [BT, D], F16, kind="ExternalOutput").ap()

    with TileContext(nc) as tc:
        with (
            tc.tile_pool(name="consts", bufs=1) as consts,
            tc.tile_pool(name="acts", bufs=1) as acts,
        ):
            # ---- resident constants / weights ----
            # q/k/v weights arrive per kt tile so the first projection matmul
            # only waits on three small DMAs. Consts issue from the gpsimd
            # queue so the sync queue can start streaming x tiles in parallel
            # (each DMA trigger costs ~0.6us of sequencer time).
            wq_ch = [consts.tile([P, QH * HD], F16, name=f"wq{i}") for i in range(KT)]
            wk_ch = [consts.tile([P, HD], F16, name=f"wk{i}") for i in range(KT)]
            wv_ch = [consts.tile([P, HD], F16, name=f"wv{i}") for i in range(KT)]
            cos_sb = consts.tile([P, T], F16)
            sin_sb = consts.tile([P, T], F16)
            perm_sb = consts.tile([P, P], F16)
            triu_sb = consts.tile([P, P], F16)
            id_sb = consts.tile([P, P], F16)
            ones_sb = consts.tile([P, P], F16)
            wo_sb = consts.tile([P, QH, D], F16)
            wq_r = wqT_d.rearrange("(a p) m -> p a m", p=P)
            wk_r = wkT_d.rearrange("(a p) m -> p a m", p=P)
            wv_r = wvT_d.rearrange("(a p) m -> p a m", p=P)

            def load_wchunk(kt, eng):
                eng.dma_start(wq_ch[kt], wq_r[:, kt, :])
                eng.dma_start(wk_ch[kt], wk_r[:, kt, :])
                eng.dma_start(wv_ch[kt], wv_r[:, kt, :])

            # kt=0 weights ride the fast sync queue ahead of the x tiles so
            # the first matmul can start asap; the rest go via gpsimd so their
            # ~0.6us-per-DMA trigger cost doesn't delay the x-tile stream.
            load_wchunk(0, nc.sync)
            for kt in range(1, 4):
                load_wchunk(kt, nc.gpsimd)
            nc.gpsimd.dma_start(cos_sb, cosT_d)
            nc.gpsimd.dma_start(sin_sb, ssinT_d)
            nc.gpsimd.dma_start(perm_sb, permT_d)
            nc.gpsimd.dma_start(id_sb, ident_d)
            for kt in range(4, KT):
                load_wchunk(kt, nc.gpsimd)
            nc.gpsimd.dma_start(triu_sb, triu_d)
            nc.gpsimd.dma_start(ones_sb, onesm_d)
            nc.gpsimd.dma_start(wo_sb, woT_d.rearrange("(h p) j -> p h j", p=P))

            # ---- resident activations ----
            qr_sb = acts.tile([P, QH, BT], F16)   # roped qT
            kr_sb = acts.tile([P, BT], F16)       # roped kT
            vt_sb = acts.tile([P, BT // P, HD], F16)  # v token-major

            # ================= phase 1: projections + rope =================
            with (
                tc.tile_pool(name="xt", bufs=6) as xt_pool,
                tc.tile_pool(name="raw", bufs=5) as raw_pool,
                tc.tile_pool(name="ropew", bufs=6) as rope_pool,
                tc.tile_pool(name="pj", bufs=5, space="PSUM") as pj,
                tc.tile_pool(name="pperm", bufs=2, space="PSUM") as pperm,
                tc.tile_pool(name="ptr", bufs=1, space="PSUM") as ptr,
            ):
                for blk in range(NBLK):
                    c0 = blk * NB          # bt column base
                    t0 = (blk % IB) * NB   # rope table base (t = bt mod T)
                    ps_q0 = pj.tile([P, NB], F32, tag="pj")
                    ps_q1 = pj.tile([P, NB], F32, tag="pj")
                    ps_k = pj.tile([P, NB], F32, tag="pj")
                    ps_v = pj.tile([P, NB], F32, tag="pj")
                    for kt in range(KT):
                        xt = xt_pool.tile([P, NB], F16, tag="xt")
                        nc.sync.dma_start(
                            xt, xT_d[kt * P:(kt + 1) * P, c0:c0 + NB]
                        )
                        st = kt == 0
                        sp = kt == KT - 1
                        nc.tensor.matmul(ps_q0, wq_ch[kt][:, 0:P], xt, start=st, stop=sp)
                        nc.tensor.matmul(ps_q1, wq_ch[kt][:, P:2 * P], xt, start=st, stop=sp)
                        nc.tensor.matmul(ps_k, wk_ch[kt], xt, start=st, stop=sp)
                        nc.tensor.matmul(ps_v, wv_ch[kt], xt, start=st, stop=sp)

                    # rope for q0, q1, k: roped = raw*cos + swap(raw)*ssin.
                    # All four PSUM drains issue first (split across Act/DVE)
                    # so the three swap matmuls then run back-to-back on the
                    # PE instead of ping-ponging with the DVE rope muls.
                    dsts = (
                        qr_sb[:, 0, c0:c0 + NB],
                        qr_sb[:, 1, c0:c0 + NB],
                        kr_sb[:, c0:c0 + NB],
                    )
                    raws = []
                    for idx, ps_raw in enumerate((ps_q0, ps_q1, ps_k)):
                        raw = raw_pool.tile([P, NB], F16, tag="raw",
                                            name=f"raw{idx}")
                        if idx == 1:
                            nc.vector.tensor_copy(raw, ps_raw)
                        else:
                            nc.scalar.copy(raw, ps_raw)
                        raws.append(raw)
                    vraw = raw_pool.tile([P, NB], F16, tag="raw")
                    nc.scalar.copy(vraw, ps_v)
                    for idx in range(3):
                        ps_sw = pperm.tile([P, NB], F32, tag="sw")
                        nc.tensor.matmul(
                            ps_sw, perm_sb, raws[idx], start=True, stop=True
                        )
                        t1 = rope_pool.tile([P, NB], F16, tag="t1")
                        nc.vector.tensor_mul(t1, raws[idx], cos_sb[:, t0:t0 + NB])
                        t2 = rope_pool.tile([P, NB], F16, tag="t2")
                        nc.vector.tensor_mul(t2, ps_sw, sin_sb[:, t0:t0 + NB])
                        nc.vector.tensor_add(dsts[idx], t1, t2)

                    # v: PE-transpose to token-major. All four transposes land
                    # in one PSUM tile so they drain with a single copy.
                    ps_t = ptr.tile([P, NB // P, P], F16, tag="tr")
                    for s in range(NB // P):
                        nc.tensor.transpose(
                            ps_t[:, s, :], vraw[:, s * P:(s + 1) * P], id_sb
                        )
                    nc.vector.tensor_copy(
                        vt_sb[:, blk * (NB // P):(blk + 1) * (NB // P), :], ps_t
                    )

            # ================= phase 2: attention + out-proj =================
            # The two local heads run interleaved j-tile by j-tile; each keeps
            # its own PSUM accumulators (y, denom). The denominator matmul uses
            # an all-ones [128,128] stationary, so every PSUM row holds the
            # row-sum -> normalize is recip_approx_fast + one multiply, no
            # partition broadcast. Out-proj runs one i-block behind attention
            # so the PE stays fed while DVE normalizes.
            with (
                tc.tile_pool(name="est", bufs=6) as est_pool,
                tc.tile_pool(name="dsum", bufs=2) as dsum_pool,
                tc.tile_pool(name="ysb", bufs=2) as y_pool,
                tc.tile_pool(name="nrm", bufs=2) as nrm_pool,
                tc.tile_pool(name="osb", bufs=4) as out_pool,
                tc.tile_pool(name="pst", bufs=2, space="PSUM") as pst,
                tc.tile_pool(name="py", bufs=2, space="PSUM") as py,
                tc.tile_pool(name="pd", bufs=2, space="PSUM") as pd,
                tc.tile_pool(name="po", bufs=2, space="PSUM") as po,
            ):
                def emit_outproj(i0p, y_prev, chunks, last=False):
                    for n in chunks:
                        s, jb = n // (D // NB), n % (D // NB)
                        row0 = i0p + s * P
                        ps_o = po.tile([P, NB], F32, tag="po")
                        for h2 in range(QH):
                            nc.tensor.matmul(
                                ps_o,
                                y_prev[:, h2, s * P:(s + 1) * P],
                                wo_sb[:, h2, jb * NB:(jb + 1) * NB],
                                start=(h2 == 0),
                                stop=(h2 == QH - 1),
                            )
                        o_sb = out_pool.tile([P, NB], F16, tag="o")
                        # alternate engines so neither queue becomes the
                        # serializer for the PSUM->SBUF drains; on the final
                        # block each engine also triggers its own DMA so the
                        # ~0.6us-per-trigger cost doesn't serialize the tail
                        # on the sync sequencer
                        orow = out_d[row0:row0 + P, jb * NB:(jb + 1) * NB]
                        if n % 2 == 1:
                            nc.vector.tensor_copy(o_sb, ps_o)
                            nc.sync.dma_start(orow, o_sb)
                        else:
                            nc.scalar.copy(o_sb, ps_o)
                            (nc.scalar if last else nc.sync).dma_start(orow, o_sb)

                pending = None
                for b in range(B):
                    cb = b * T  # bt base of this batch
                    for ib in range(IB):
                        i0 = cb + ib * NB  # global bt col base of q block
                        jt_max = 4 * ib + 3
                        y_sb = y_pool.tile([P, QH, NB], F16, tag="y")
                        ps_y = [
                            py.tile([P, NB], F32, tag="py", name=f"psy{h}")
                            for h in range(QH)
                        ]
                        dsum = [
                            dsum_pool.tile([P, NB], F16, tag="ds", name=f"ds{h}")
                            for h in range(QH)
                        ]
                        for jt in range(jt_max + 1):
                            a = jt - 4 * ib
                            sub = max(0, a) * P
                            ests = []
                            for h in range(QH):
                                ps_s = pst.tile([P, NB], F32, tag="st")
                                nc.tensor.matmul(
                                    ps_s[:, sub:],
                                    kr_sb[:, cb + jt * P:cb + (jt + 1) * P],
                                    qr_sb[:, h, i0 + sub:i0 + NB],
                                    start=True,
                                    stop=True,
                                )
                                est = est_pool.tile([P, NB], F16, tag="est")
                                nc.scalar.activation(
                                    est[:, sub:], ps_s[:, sub:], EXP, scale=SCALE
                                )
                                if a >= 0:  # diagonal tile: tri mask
                                    nc.vector.tensor_mul(
                                        est[:, sub:sub + P],
                                        est[:, sub:sub + P],
                                        triu_sb,
                                    )
                                # running denominator sum on DVE (partition
                                # reduction happens once per block below)
                                if jt == 0:
                                    nc.vector.tensor_copy(dsum[h], est)
                                else:
                                    nc.vector.tensor_add(
                                        dsum[h][:, sub:],
                                        dsum[h][:, sub:],
                                        est[:, sub:],
                                    )
                                ests.append(est)
                            st_f = jt == 0
                            sp_f = jt == jt_max
                            for h in range(QH):
                                nc.tensor.matmul(
                                    ps_y[h][:, sub:],
                                    vt_sb[:, (cb // P) + jt, :],
                                    ests[h][:, sub:],
                                    start=st_f,
                                    stop=sp_f,
                                )
                        def normalize():
                            for h in range(QH):
                                # ones @ dsum: every PSUM row = the denom row
                                ps_d = pd.tile([P, NB], F32, tag="pd",
                                               name=f"psd{h}")
                                nc.tensor.matmul(
                                    ps_d, ones_sb, dsum[h], start=True, stop=True
                                )
                                rf = nrm_pool.tile([P, NB], F32, tag="rf")
                                nc.vector.reciprocal_approx_fast(rf, ps_d)
                                nc.vector.tensor_mul(y_sb[:, h, :], ps_y[h], rf)

                        # normalize after a short out-proj prefix: the
                        # denominator adds finish under out-proj cover, and
                        # recip/ymul complete before the next block's first
                        # PV matmul needs the ps_y banks back
                        if pending is not None:
                            emit_outproj(*pending, chunks=range(4))
                            normalize()
                            emit_outproj(*pending, chunks=range(4, 16))
                        else:
                            normalize()
                        pending = (i0, y_sb)
                emit_outproj(*pending, chunks=range(16), last=True)

    nc.compile()
    return nc


def _host_prep(x, rope, wq, wk, wv, wo):
    """Build the 8 per-core input maps (shard + pre-transpose on host)."""
    f16 = np.float16
    xT = np.ascontiguousarray(x.reshape(BT, D).T.astype(f16))
    cos = np.asarray(rope[..., 0], dtype=np.float32)  # [T, 64]
    sin = np.asarray(rope[..., 1], dtype=np.float32)
    cosT = np.ascontiguousarray(np.concatenate([cos.T, cos.T], axis=0).astype(f16))
    ssinT = np.ascontiguousarray(np.concatenate([-sin.T, sin.T], axis=0).astype(f16))
    perm = np.zeros((P, P), dtype=f16)
    perm[(np.arange(P) + 64) % P, np.arange(P)] = 1.0
    triu = np.triu(np.ones((P, P), dtype=f16))
    ident = np.eye(P, dtype=f16)
    onesm = np.ones((P, P), dtype=f16)

    in_maps = []
    for c in range(NCORES):
        kv = c // 2
        in_maps.append(
            {
                "xT": xT,
                "wqT": np.ascontiguousarray(
                    wq[QH * HD * c:QH * HD * (c + 1), :].T.astype(f16)
                ),
                "wkT": np.ascontiguousarray(wk[HD * kv:HD * (kv + 1), :].T.astype(f16)),
                "wvT": np.ascontiguousarray(wv[HD * kv:HD * (kv + 1), :].T.astype(f16)),
                "woT": np.ascontiguousarray(
                    wo[:, QH * HD * c:QH * HD * (c + 1)].T.astype(f16)
                ),
                "cosT": cosT,
                "ssinT": ssinT,
                "permT": perm,
                "triu": triu,
                "ident": ident,
                "onesm": onesm,
            }
        )
    return in_maps


LAST_RESULTS = None


def kernel(x, rope, wq, wk, wv, wo):
    global LAST_RESULTS
    from concourse import bass_utils

    if "nc" not in _CACHE:
        _CACHE["nc"] = _build()
    nc = _CACHE["nc"]

    in_maps = _host_prep(
        np.asarray(x), np.asarray(rope), np.asarray(wq), np.asarray(wk), np.asarray(wv),
        np.asarray(wo)
    )
    res = bass_utils.run_bass_kernel_spmd(nc, in_maps, core_ids=list(range(NCORES)))
    LAST_RESULTS = res
    acc = np.zeros((BT, D), dtype=np.float32)
    for c in range(NCORES):
        acc += res.results[c]["out"].astype(np.float32)
    return acc.reshape(B, T, D)


# revision 42
# speedup vs baseline: 1.0391x; 1.0391x over previous
"""GQA attention forward on 8 TRN2 NeuronCores, tensor-parallel across heads.

Problem (hardcoded): B=2, T=2048, D=2048, 16 q-heads, 4 kv-heads, head_dim=128,
RoPE (rotate-half pairing i <-> i+64), causal softmax, output projection.

Sharding (per core c of 8):
  q-heads 2c, 2c+1 (rows 256c:256c+256 of wq), kv-head c//2 (rows of wk/wv),
  wo input-dim slice [:, 256c:256c+256]. x replicated. Each core computes a
  full-shape partial of the output (y_local @ wo_slice.T); host sums partials.

On-core layout: activations kept feature-major (qT/kT = [head_dim, tokens]) so
every matmul contraction dim lands on SBUF partitions with zero transposes of
x (host pre-transposes x once). Scores are computed transposed (ST[j,i]) so
P@V needs no transpose either. All matmul operands are fp16 (full PE speed,
half the LDWEIGHTS/DMA/SBUF cost of fp32r, ~5e-4 matmul rel err); PSUM
accumulation stays fp32. Max-subtraction is skipped (scores are O(5),
exp(s) < 100 is fp16-safe). Softmax denominators: est tiles are summed on the
vector engine (dsum) and reduced over partitions once per block by a
ones-MATRIX matmul whose output rows are the broadcast row-sum, so no
partition broadcast is needed; 1/d uses reciprocal_approx_fast. The two local
heads interleave tile-by-tile; each block's normalize is sandwiched after a
4-chunk prefix of the previous block's out-proj so the PE never waits on the
denominator-add chain, and recip/ymul retire before the next block needs the
ps_y banks. Weight loads are chunked per kt and issued from the gpsimd queue
(consts) in parallel with x tiles on sync; PSUM->SBUF drains alternate
Act/DVE so neither in-order queue serializes the pipeline.
"""
import math
import numpy as np

P = 128
B = 2
T = 2048
D = 2048
BT = B * T            # 4096
HD = 128              # head dim
QH = 2                # local q heads per core
KT = D // P           # 16 contraction tiles over D
NB = 512              # free-dim block (tokens) for matmuls
NBLK = BT // NB       # 8 bt blocks
IB = T // NB          # 4 i-blocks per batch
NCORES = 8
SCALE = 1.0 / math.sqrt(HD)

_CACHE = {}


def _build():
    import concourse.bass as bass
    import concourse.mybir as mybir
    from concourse import bacc
    from concourse.tile import TileContext

    F32 = mybir.dt.float32
    F16 = mybir.dt.float16
    EXP = mybir.ActivationFunctionType.Exp

    nc = bacc.Bacc("TRN2", target_bir_lowering=False, debug=False)

    xT_d = nc.dram_tensor("xT", [D, BT], F16, kind="ExternalInput").ap()
    wqT_d = nc.dram_tensor("wqT", [D, QH * HD], F16, kind="ExternalInput").ap()
    wkT_d = nc.dram_tensor("wkT", [D, HD], F16, kind="ExternalInput").ap()
    wvT_d = nc.dram_tensor("wvT", [D, HD], F16, kind="ExternalInput").ap()
    woT_d = nc.dram_tensor("woT", [QH * HD, D], F16, kind="ExternalInput").ap()
    cosT_d = nc.dram_tensor("cosT", [P, T], F16, kind="ExternalInput").ap()
    ssinT_d = nc.dram_tensor("ssinT", [P, T], F16, kind="ExternalInput").ap()
    permT_d = nc.dram_tensor("permT", [P, P], F16, kind="ExternalInput").ap()
    triu_d = nc.dram_tensor("triu", [P, P], F16, kind="ExternalInput").ap()
    ident_d = nc.dram_tensor("ident", [P, P], F16, kind="ExternalInput").ap()
    onesm_d = nc.dram_tensor("onesm", [P, P], F16, kind="ExternalInput").ap()
    out_d = nc.dram_tensor("out", [BT, D], F16, kind="ExternalOutput").ap()

    with TileContext(nc) as tc:
        with (
            tc.tile_pool(name="consts", bufs=1) as consts,
            tc.tile_pool(name="acts", bufs=1) as acts,
        ):
            # ---- resident constants / weights ----
            # q/k/v weights arrive per kt tile so the first projection matmul
            # only waits on three small DMAs. Consts issue from the gpsimd
            # queue so the sync queue can start streaming x tiles in parallel
            # (each DMA trigger costs ~0.6us of sequencer time).
            wq_ch = [consts.tile([P, QH * HD], F16, name=f"wq{i}") for i in range(KT)]
            wk_ch = [consts.tile([P, HD], F16, name=f"wk{i}") for i in range(KT)]
            wv_ch = [consts.tile([P, HD], F16, name=f"wv{i}") for i in range(KT)]
            cos_sb = consts.tile([P, T], F16)
            sin_sb = consts.tile([P, T], F16)
            perm_sb = consts.tile([P, P], F16)
            triu_sb = consts.tile([P, P], F16)
            id_sb = consts.tile([P, P], F16)
            ones_sb = consts.tile([P, P], F16)
            wo_sb = consts.tile([P, QH, D], F16)
            wq_r = wqT_d.rearrange("(a p) m -> p a m", p=P)
            wk_r = wkT_d.rearrange("(a p) m -> p a m", p=P)
            wv_r = wvT_d.rearrange("(a p) m -> p a m", p=P)

            def load_wchunk(kt, eng):
                eng.dma_start(wq_ch[kt], wq_r[:, kt, :])
                eng.dma_start(wk_ch[kt], wk_r[:, kt, :])
                eng.dma_start(wv_ch[kt], wv_r[:, kt, :])

            # kt=0 weights ride the fast sync queue ahead of the x tiles so
            # the first matmul can start asap; the rest go via gpsimd so their
            # ~0.6us-per-DMA trigger cost doesn't delay the x-tile stream.
            load_wchunk(0, nc.sync)
            for kt in range(1, 4):
                load_wchunk(kt, nc.gpsimd)
            nc.gpsimd.dma_start(cos_sb, cosT_d)
            nc.gpsimd.dma_start(sin_sb, ssinT_d)
            nc.gpsimd.dma_start(perm_sb, permT_d)
            nc.gpsimd.dma_start(id_sb, ident_d)
            for kt in range(4, KT):
                load_wchunk(kt, nc.gpsimd)
            nc.gpsimd.dma_start(triu_sb, triu_d)
            nc.gpsimd.dma_start(ones_sb, onesm_d)
            nc.gpsimd.dma_start(wo_sb, woT_d.rearrange("(h p) j -> p h j", p=P))

            # ---- resident activations ----
            qr_sb = acts.tile([P, QH, BT], F16)   # roped qT
            kr_sb = acts.tile([P, BT], F16)       # roped kT
            vt_sb = acts.tile([P, BT // P, HD], F16)  # v token-major

            # ================= phase 1: projections + rope =================
            with (
                tc.tile_pool(name="xt", bufs=6) as xt_pool,
                tc.tile_pool(name="raw", bufs=5) as raw_pool,
                tc.tile_pool(name="ropew", bufs=6) as rope_pool,
                tc.tile_pool(name="pj", bufs=6, space="PSUM") as pj,
                tc.tile_pool(name="pperm", bufs=1, space="PSUM") as pperm,
                tc.tile_pool(name="ptr", bufs=1, space="PSUM") as ptr,
            ):
                for blk in range(NBLK):
                    c0 = blk * NB          # bt column base
                    t0 = (blk % IB) * NB   # rope table base (t = bt mod T)
                    ps_q0 = pj.tile([P, NB], F32, tag="pj")
                    ps_q1 = pj.tile([P, NB], F32, tag="pj")
                    ps_k = pj.tile([P, NB], F32, tag="pj")
                    ps_v = pj.tile([P, NB], F32, tag="pj")
                    for kt in range(KT):
                        xt = xt_pool.tile([P, NB], F16, tag="xt")
                        nc.sync.dma_start(
                            xt, xT_d[kt * P:(kt + 1) * P, c0:c0 + NB]
                        )
                        st = kt == 0
                        sp = kt == KT - 1
                        nc.tensor.matmul(ps_q0, wq_ch[kt][:, 0:P], xt, start=st, stop=sp)
                        nc.tensor.matmul(ps_q1, wq_ch[kt][:, P:2 * P], xt, start=st, stop=sp)
                        nc.tensor.matmul(ps_k, wk_ch[kt], xt, start=st, stop=sp)
                        nc.tensor.matmul(ps_v, wv_ch[kt], xt, start=st, stop=sp)

                    # rope for q0, q1, k: roped = raw*cos + swap(raw)*ssin.
                    # All four PSUM drains issue first (split across Act/DVE)
                    # so the three swap matmuls then run back-to-back on the
                    # PE instead of ping-ponging with the DVE rope muls.
                    dsts = (
                        qr_sb[:, 0, c0:c0 + NB],
                        qr_sb[:, 1, c0:c0 + NB],
                        kr_sb[:, c0:c0 + NB],
                    )
                    raws = []
                    for idx, ps_raw in enumerate((ps_q0, ps_q1, ps_k)):
                        raw = raw_pool.tile([P, NB], F16, tag="raw",
                                            name=f"raw{idx}")
                        if idx == 1:
                            nc.vector.tensor_copy(raw, ps_raw)
                        else:
                            nc.scalar.copy(raw, ps_raw)
                        raws.append(raw)
                    vraw = raw_pool.tile([P, NB], F16, tag="raw")
                    nc.scalar.copy(vraw, ps_v)
                    for idx in range(3):
                        ps_sw = pperm.tile([P, NB], F32, tag="sw")
                        nc.tensor.matmul(
                            ps_sw, perm_sb, raws[idx], start=True, stop=True
                        )
                        t1 = rope_pool.tile([P, NB], F16, tag="t1")
                        nc.vector.tensor_mul(t1, raws[idx], cos_sb[:, t0:t0 + NB])
                        t2 = rope_pool.tile([P, NB], F16, tag="t2")
                        nc.vector.tensor_mul(t2, ps_sw, sin_sb[:, t0:t0 + NB])
                        nc.vector.tensor_add(dsts[idx], t1, t2)

                    # v: PE-transpose to token-major. All four transposes land
                    # in one PSUM tile so they drain with a single copy.
                    ps_t = ptr.tile([P, NB // P, P], F16, tag="tr")
                    for s in range(NB // P):
                        nc.tensor.transpose(
                            ps_t[:, s, :], vraw[:, s * P:(s + 1) * P], id_sb
                        )
                    nc.vector.tensor_copy(
                        vt_sb[:, blk * (NB // P):(blk + 1) * (NB // P), :], ps_t
                    )

            # ================= phase 2: attention + out-proj =================
            # The two local heads run interleaved j-tile by j-tile; each keeps
            # its own PSUM accumulators (y, denom). The denominator matmul uses
            # an all-ones [128,128] stationary, so every PSUM row holds the
            # row-sum -> normalize is recip_approx_fast + one multiply, no
            # partition broadcast. Out-proj runs one i-block behind attention
            # so the PE stays fed while DVE normalizes.
            with (
                tc.tile_pool(name="est", bufs=6) as est_pool,
                tc.tile_pool(name="dsum", bufs=2) as dsum_pool,
                tc.tile_pool(name="ysb", bufs=2) as y_pool,
                tc.tile_pool(name="nrm", bufs=2) as nrm_pool,
                tc.tile_pool(name="osb", bufs=4) as out_pool,
                tc.tile_pool(name="pst", bufs=2, space="PSUM") as pst,
                tc.tile_pool(name="py", bufs=2, space="PSUM") as py,
                tc.tile_pool(name="pd", bufs=2, space="PSUM") as pd,
                tc.tile_pool(name="po", bufs=2, space="PSUM") as po,
            ):
                def emit_outproj(i0p, y_prev, chunks, act_every=2, last=False):
                    for n in chunks:
                        s, jb = n // (D // NB), n % (D // NB)
                        row0 = i0p + s * P
                        ps_o = po.tile([P, NB], F32, tag="po")
                        for h2 in range(QH):
                            nc.tensor.matmul(
                                ps_o,
                                y_prev[:, h2, s * P:(s + 1) * P],
                                wo_sb[:, h2, jb * NB:(jb + 1) * NB],
                                start=(h2 == 0),
                                stop=(h2 == QH - 1),
                            )
                        o_sb = out_pool.tile([P, NB], F16, tag="o")
                        # split drains across Act/DVE so neither in-order
                        # queue serializes; long attention blocks are
                        # Act-bound on exps, so they give Act fewer copies.
                        # On the final block Act triggers its own DMAs so the
                        # ~0.6us-per-trigger cost leaves the sync sequencer.
                        orow = out_d[row0:row0 + P, jb * NB:(jb + 1) * NB]
                        if n % act_every != 0:
                            nc.vector.tensor_copy(o_sb, ps_o)
                            nc.sync.dma_start(orow, o_sb)
                        else:
                            nc.scalar.copy(o_sb, ps_o)
                            (nc.scalar if last else nc.sync).dma_start(orow, o_sb)

                pending = None
                for b in range(B):
                    cb = b * T  # bt base of this batch
                    for ib in range(IB):
                        i0 = cb + ib * NB  # global bt col base of q block
                        jt_max = 4 * ib + 3
                        y_sb = y_pool.tile([P, QH, NB], F16, tag="y")
                        ps_y = [
                            py.tile([P, NB], F32, tag="py", name=f"psy{h}")
                            for h in range(QH)
                        ]
                        dsum = [
                            dsum_pool.tile([P, NB], F16, tag="ds", name=f"ds{h}")
                            for h in range(QH)
                        ]
                        for jt in range(jt_max + 1):
                            a = jt - 4 * ib
                            sub = max(0, a) * P
                            ests = []
                            for h in range(QH):
                                ps_s = pst.tile([P, NB], F32, tag="st")
                                nc.tensor.matmul(
                                    ps_s[:, sub:],
                                    kr_sb[:, cb + jt * P:cb + (jt + 1) * P],
                                    qr_sb[:, h, i0 + sub:i0 + NB],
                                    start=True,
                                    stop=True,
                                )
                                est = est_pool.tile([P, NB], F16, tag="est")
                                nc.scalar.activation(
                                    est[:, sub:], ps_s[:, sub:], EXP, scale=SCALE
                                )
                                if a >= 0:  # diagonal tile: tri mask
                                    nc.vector.tensor_mul(
                                        est[:, sub:sub + P],
                                        est[:, sub:sub + P],
                                        triu_sb,
                                    )
                                # running denominator sum on DVE (partition
                                # reduction happens once per block below)
                                if jt == 0:
                                    nc.vector.tensor_copy(dsum[h], est)
                                else:
                                    nc.vector.tensor_add(
                                        dsum[h][:, sub:],
                                        dsum[h][:, sub:],
                                        est[:, sub:],
                                    )
                                ests.append(est)
                            st_f = jt == 0
                            sp_f = jt == jt_max
                            for h in range(QH):
                                nc.tensor.matmul(
                                    ps_y[h][:, sub:],
                                    vt_sb[:, (cb // P) + jt, :],
                                    ests[h][:, sub:],
                                    start=st_f,
                                    stop=sp_f,
                                )
                        def normalize():
                            for h in range(QH):
                                # ones @ dsum: every PSUM row = the denom row
                                ps_d = pd.tile([P, NB], F32, tag="pd",
                                               name=f"psd{h}")
                                nc.tensor.matmul(
                                    ps_d, ones_sb, dsum[h], start=True, stop=True
                                )
                                rf = nrm_pool.tile([P, NB], F32, tag="rf")
                                nc.vector.reciprocal_approx_fast(rf, ps_d)
                                nc.vector.tensor_mul(y_sb[:, h, :], ps_y[h], rf)

                        # normalize after a short out-proj prefix: the
                        # denominator adds finish under out-proj cover, and
                        # recip/ymul complete before the next block's first
                        # PV matmul needs the ps_y banks back
                        ae = 2 if ib < 2 else (3 if ib == 2 else 4)
                        if pending is not None:
                            emit_outproj(*pending, chunks=range(4), act_every=ae)
                            normalize()
                            emit_outproj(*pending, chunks=range(4, 16),
                                         act_every=ae)
                        else:
                            normalize()
                        pending = (i0, y_sb)
                emit_outproj(*pending, chunks=range(16), last=True)

    nc.compile()
    return nc


def _host_prep(x, rope, wq, wk, wv, wo):
    """Build the 8 per-core input maps (shard + pre-transpose on host)."""
    f16 = np.float16
    xT = np.ascontiguousarray(x.reshape(BT, D).T.astype(f16))
    cos = np.asarray(rope[..., 0], dtype=np.float32)  # [T, 64]
    sin = np.asarray(rope[..., 1], dtype=np.float32)
    cosT = np.ascontiguousarray(np.concatenate([cos.T, cos.T], axis=0).astype(f16))
    ssinT = np.ascontiguousarray(np.concatenate([-sin.T, sin.T], axis=0).astype(f16))
    perm = np.zeros((P, P), dtype=f16)
    perm[(np.arange(P) + 64) % P, np.arange(P)] = 1.0
    triu = np.triu(np.ones((P, P), dtype=f16))
    ident = np.eye(P, dtype=f16)
    onesm = np.ones((P, P), dtype=f16)

    in_maps = []
    for c in range(NCORES):
        kv = c // 2
        in_maps.append(
            {
                "xT": xT,
                "wqT": np.ascontiguousarray(
                    wq[QH * HD * c:QH * HD * (c + 1), :].T.astype(f16)
                ),
                "wkT": np.ascontiguousarray(wk[HD * kv:HD * (kv + 1), :].T.astype(f16)),
                "wvT": np.ascontiguousarray(wv[HD * kv:HD * (kv + 1), :].T.astype(f16)),
                "woT": np.ascontiguousarray(
                    wo[:, QH * HD * c:QH * HD * (c + 1)].T.astype(f16)
                ),
                "cosT": cosT,
                "ssinT": ssinT,
                "permT": perm,
                "triu": triu,
                "ident": ident,
                "onesm": onesm,
            }
        )
    return in_maps


LAST_RESULTS = None


def kernel(x, rope, wq, wk, wv, wo):
    global LAST_RESULTS
    from concourse import bass_utils

    if "nc" not in _CACHE:
        _CACHE["nc"] = _build()
    nc = _CACHE["nc"]

    in_maps = _host_prep(
        np.asarray(x), np.asarray(rope), np.asarray(wq), np.asarray(wk), np.asarray(wv),
        np.asarray(wo)
    )
    res = bass_utils.run_bass_kernel_spmd(nc, in_maps, core_ids=list(range(NCORES)))
    LAST_RESULTS = res
    acc = np.zeros((BT, D), dtype=np.float32)
    for c in range(NCORES):
        acc += res.results[c]["out"].astype(np.float32)
    return acc.reshape(B, T, D)
